# revision 45
# baseline (speedup 1.0000x reference)
"""Trainium2 Bass kernel for LGRL classifier decoder (segment softmax-pool MLP).

Math (reference):
    extra = io_embed.reshape(B, Y)[segment_ids]                # (T, Y)
    h1 = relu([ps_data, extra] @ W1 + b1)
    h2 = relu(h1 @ W2 + b2)
    logits = (h2 @ W3 + b3)[:, 0]
    w = segment_softmax(logits)
    pooled = segment_sum(w * ps_data)                          # (B, X)
    out = relu(pooled @ Wf1 + bf1) @ Wf2 + bf2                 # (B, 2)

Key transformations:
  * segment-aligned sharding: segment_ids are sorted, so the host assigns
    core c ALL tokens of segments [8c, 8c+8), padded to a fixed tloc with
    dummy tokens whose one-hot columns are zero (exact no-op in every
    reduction).  All segment reductions become core-local: NO collectives.
  * [ps, extra] @ W1 = ps @ W1a + onehot8(seg) @ (io8 @ W1b + b1): the
    extra-part matmul collapses to a (8, Y) @ (Y, H) on-device precompute
    plus a rank-8 one-hot matmul; with K=8 the four hc-chunk matmuls run
    CONCURRENTLY in four 32-row groups of the PE array (row tiling).
  * pooling scales the 8-col one-hot by e and runs the four subtile
    matmuls concurrently in four 32-partition output groups (col tiling);
    a final select-matrix matmul folds the four groups.
  * per-segment max subtraction in the softmax is dropped (shift
    invariant, logits are O(0.1)); b3 dropped for the same reason.
  * the host ships ps twice: token-major bf16 (pool path) and
    feature-major fp8 (MLP path), pre-arranged to the exact SBUF layout:
    no PE transposes, no dtype-converting DMAs.
  * h1/h2/logits matmuls run fp8 DoubleRow (measured ~216ns per
    FD=512 matmul with LDWEIGHTS fully hidden), fp32 PSUM accumulation.
"""

import numpy as np
import ml_dtypes

import concourse.bass as bass
import concourse.mybir as mybir
import concourse.tile as tile
from concourse import bacc
from concourse.bass_utils import run_bass_kernel_spmd
from concourse.masks import make_identity

B = 64
T = 65536
X = 512
KIO = 5
Y = X * KIO          # 2560
H = 512
NCORES = 8
P = 128
FP32 = mybir.dt.float32
BF16 = mybir.dt.bfloat16
FP8 = mybir.dt.float8e4
AF = mybir.ActivationFunctionType
ALU = mybir.AluOpType

KC = X // P          # 4 contraction chunks for 512-dims
HC = H // P          # 4 output chunks for 512-dims
NKB = Y // P         # 20 contraction chunks of W1b
MT = 512             # tokens per MLP tile
NSUB = MT // P       # 128-token subtiles per MLP tile
BL = B // NCORES     # segments owned per core (local)
TLOC = 8704          # per-core padded token count (8192 + 512 slack)


def build(tloc=TLOC):
    """Build + compile the SPMD kernel for per-core token count `tloc`."""
    nt = tloc // MT
    nc = bacc.Bacc(
        "TRN2", target_bir_lowering=False, debug=False, num_devices=NCORES
    )

    psT = nc.dram_tensor("psT", [nt, P, KC, MT], FP8, kind="ExternalInput").ap()
    # ps carries the 8-col local one-hot in columns X:X+BL (one DMA/tile)
    ps = nc.dram_tensor("ps", [nt, P, NSUB, X + BL], BF16, kind="ExternalInput").ap()
    st4 = nc.dram_tensor("st4", [P, tloc], BF16, kind="ExternalInput").ap()
    ioT = nc.dram_tensor("ioT", [P, NKB, BL], BF16, kind="ExternalInput").ap()
    w1b = nc.dram_tensor("w1b", [P, NKB, H], BF16, kind="ExternalInput").ap()
    b1 = nc.dram_tensor("b1", [1, H], BF16, kind="ExternalInput").ap()
    w1a = nc.dram_tensor("w1a", [P, KC, H], FP8, kind="ExternalInput").ap()
    w2 = nc.dram_tensor("w2", [P, KC, H], FP8, kind="ExternalInput").ap()
    b2 = nc.dram_tensor("b2", [P, HC], FP32, kind="ExternalInput").ap()
    w3 = nc.dram_tensor("w3", [P, KC, 1], FP8, kind="ExternalInput").ap()
    sel = nc.dram_tensor("sel", [P, BL], BF16, kind="ExternalInput").ap()
    rep4 = nc.dram_tensor("rep4", [BL, P], BF16, kind="ExternalInput").ap()
    wf1 = nc.dram_tensor("wf1", [P, KC, H], BF16, kind="ExternalInput").ap()
    bf1_t = nc.dram_tensor("bf1", [P, HC], FP32, kind="ExternalInput").ap()
    wf2 = nc.dram_tensor("wf2", [P, KC, 2], BF16, kind="ExternalInput").ap()
    bf2_t = nc.dram_tensor("bf2", [2, 1], FP32, kind="ExternalInput").ap()
    outT = nc.dram_tensor("outT", [2, BL], FP32, kind="ExternalOutput").ap()

    with tile.TileContext(nc) as tc:
        with (
            tc.tile_pool(name="const", bufs=1) as cpool,
            tc.tile_pool(name="work", bufs=2) as wpool,
            tc.tile_pool(name="psum", bufs=1, space="PSUM") as ppool,
        ):
            # ---------------- constants ----------------
            identf = cpool.tile([1, 1], FP32)
            nc.gpsimd.memset(identf, 1.0)
            ones_b = cpool.tile([1, BL], BF16)
            nc.gpsimd.memset(ones_b, 1.0)
            ones_col = cpool.tile([P, 1], BF16)
            nc.gpsimd.memset(ones_col, 1.0)
            identbr = cpool.tile([BL, BL], FP32)
            make_identity(nc, identbr)

            # DMA issue order is tuned so the first ~10us keeps every queue
            # streaming what the PE consumes first: ps0 (pool of tile 0) on
            # sync ahead of w1b's sync share; w1b split scalar/sync feeding
            # the seg-block matmuls; st4's first chunk on gpsimd.
            def _tile_dma(j, ps_eng=None):
                psT_sb = wpool.tile(
                    [P, KC, MT], FP8, tag="psT", bufs=6, name=f"psT_{j}"
                )
                (nc.scalar if j % 2 == 0 else nc.sync).dma_start(psT_sb, psT[j])
                ps_sb = wpool.tile(
                    [P, NSUB, X + BL], BF16, tag="ps", bufs=7, name=f"ps_{j}"
                )
                (ps_eng or nc.gpsimd).dma_start(ps_sb, ps[j])
                return psT_sb, ps_sb

            ps0_sb = wpool.tile(
                [P, NSUB, X + BL], BF16, tag="ps", bufs=7, name="ps_0"
            )
            nc.sync.dma_start(ps0_sb, ps[0])

            ioT_sb = cpool.tile([P, NKB, BL], BF16)
            nc.scalar.dma_start(ioT_sb, ioT)
            rep4_sb = cpool.tile([BL, P], BF16)
            nc.scalar.dma_start(rep4_sb, rep4)
            w1b_sb = wpool.tile([P, NKB, H], BF16, tag="w1b", bufs=1)
            Q = NKB // 4
            for q in range(4):
                eng = nc.scalar if q < 2 else nc.sync
                eng.dma_start(
                    w1b_sb[:, q * Q : (q + 1) * Q, :], w1b[:, q * Q : (q + 1) * Q, :]
                )
            b1_sb = cpool.tile([1, H], BF16)
            nc.scalar.dma_start(b1_sb, b1)
            w1a_sb = cpool.tile([P, KC, H], FP8)
            nc.scalar.dma_start(w1a_sb, w1a)

            psT0_sb = wpool.tile([P, KC, MT], FP8, tag="psT", bufs=6, name="psT_0")
            nc.scalar.dma_start(psT0_sb, psT[0])
            pre = [(psT0_sb, ps0_sb)]

            st4_sb = cpool.tile([P, tloc], BF16)
            CH = tloc // 4
            nc.gpsimd.dma_start(st4_sb[:, 0:CH], st4[:, 0:CH])
            NPRE = min(3, nt)
            for j in range(1, NPRE):
                pre.append(_tile_dma(j))
            nc.sync.dma_start(st4_sb[:, CH : 2 * CH], st4[:, CH : 2 * CH])

            w2_sb = cpool.tile([P, KC, H], FP8)
            nc.scalar.dma_start(w2_sb, w2)
            # 16-wide w3 tile keeps the DoubleRow pair step a multiple of 16B
            w3_sb = cpool.tile([P, KC, 16], FP8)
            nc.scalar.dma_start(w3_sb[:, :, 0:1], w3)
            b2_sb = cpool.tile([P, HC], FP32)
            nc.scalar.dma_start(b2_sb, b2)
            sel_sb = cpool.tile([P, BL], BF16)
            nc.scalar.dma_start(sel_sb, sel)
            # late st4 chunks ride the gpsimd queue behind the early ps
            # tiles (needed only from tile ~nt/2 on) so they never delay psT
            nc.gpsimd.dma_start(st4_sb[:, 2 * CH : 3 * CH], st4[:, 2 * CH : 3 * CH])
            nc.gpsimd.dma_start(st4_sb[:, 3 * CH :], st4[:, 3 * CH :])
            # final-fc weights are only needed at the tail; allocate now,
            # DMA later (emitted just before the finalize section)
            wf1_sb = cpool.tile([P, KC, H], BF16)
            bf1_sb = cpool.tile([P, HC], FP32)
            wf2_sb = cpool.tile([P, KC, 2], BF16)
            bf2_sb = cpool.tile([2, 1], FP32)

            # ---------------- seg8 = io8 @ W1b + b1  (BL, H) ----------------
            seg_psum = ppool.tile([P, H], FP32, tag="poolacc", bufs=1)
            for kb in range(NKB):
                nc.tensor.matmul(
                    seg_psum[0:BL, :],
                    ioT_sb[:, kb, :],
                    w1b_sb[:, kb, :],
                    start=(kb == 0),
                    stop=False,
                )
            nc.tensor.matmul(
                seg_psum[0:BL, :], ones_b, b1_sb, start=False, stop=True
            )
            seg_sb = cpool.tile([BL, H], BF16)
            nc.vector.tensor_copy(seg_sb, seg_psum[0:BL, :])
            # replicate seg8 into the four 32-row groups via one select
            # matmul (avoids queue-blocking SBUF->SBUF DMAs)
            NR = 3 * 32 + BL  # 104 rows cover all four groups
            seg_rep = ppool.tile([P, H], FP32, tag="lp", bufs=1)
            nc.tensor.matmul(
                seg_rep[0:NR, :], rep4_sb[:, 0:NR], seg_sb, start=True, stop=True
            )
            seg_dup = cpool.tile([P, H], BF16)
            nc.vector.tensor_copy(seg_dup[0:NR, :], seg_rep[0:NR, :])

            # ---------------- main loop over MLP tiles ----------------
            # pool regions: subtile s accumulates into partitions
            # [32s, 32s+BL); a final sel-matmul folds the four regions.
            pool_psum = ppool.tile([P, H], FP32, tag="poolacc", bufs=1)
            den_psum = ppool.tile([1, NSUB * BL], FP32, tag="den", bufs=1)
            prev = None  # (j, ps_sb, e_row) of previous tile
            pending = []  # [(j, ps_sb, stmw)] awaiting pool matmuls

            def emit_e(pj, p_ps, p_erow):
                eTp = ppool.tile([P, NSUB], FP32, tag="eT", bufs=1)
                for s in range(NSUB):
                    nc.tensor.transpose(
                        eTp[:, s : s + 1],
                        p_erow[0:1, s * P : (s + 1) * P],
                        identf[0:1, 0:1],
                    )
                e_col = wpool.tile([P, NSUB], FP32, tag="ecol", bufs=2)
                nc.vector.tensor_copy(e_col, eTp)
                stmw = wpool.tile([P, NSUB, BL], BF16, tag="stmw", bufs=5)
                for s in range(NSUB):
                    nc.vector.tensor_scalar_mul(
                        stmw[:, s, :],
                        p_ps[:, s, X : X + BL],
                        e_col[:, s : s + 1],
                    )
                return stmw

            def emit_pool(pj, p_ps, stmw):
                first = pj == 0
                last = pj == nt - 1
                for s in range(NSUB):
                    nc.tensor.matmul(
                        pool_psum[32 * s : 32 * s + BL, :],
                        stmw[:, s, :],
                        p_ps[:, s, 0:X],
                        start=first,
                        stop=last,
                        tile_position=(0, 32 * s),
                        skip_group_check=True,
                    )
                nc.tensor.matmul(
                    den_psum[0:1, :],
                    ones_col,
                    stmw[:, :, :],
                    start=first,
                    stop=last,
                )

            for j in range(nt):
                psT_sb, ps_sb = pre[j] if j < NPRE else _tile_dma(j)
                if j == 10:
                    # final-fc weights: issued mid-loop on gpsimd so they
                    # land well before the tail without delaying any psT
                    nc.gpsimd.dma_start(wf1_sb, wf1)
                    nc.gpsimd.dma_start(bf1_sb, bf1_t)
                    nc.gpsimd.dma_start(wf2_sb, wf2)
                    nc.gpsimd.dma_start(bf2_sb, bf2_t)

                # h1 = relu(psT-major matmuls + seg8 one-hot broadcast);
                # the four K=8 seg matmuls run concurrently in four 32-row
                # groups of the PE array.
                h1_sb = wpool.tile([P, KC, MT], FP8, tag="h1", bufs=2)
                h1ps = []
                for hc in range(HC):
                    h1p = ppool.tile([P, MT], FP32, tag="hp", bufs=4)
                    for kc in range(0, KC, 2):
                        nc.tensor.matmul(
                            h1p,
                            w1a_sb[:, kc : kc + 2, hc * P : (hc + 1) * P],
                            psT_sb[:, kc : kc + 2, :],
                            start=(kc == 0),
                            stop=False,
                            perf_mode=mybir.MatmulPerfMode.DoubleRow,
                        )
                    h1ps.append(h1p)
                for hc in range(HC):
                    nc.tensor.matmul(
                        h1ps[hc],
                        seg_dup[32 * hc : 32 * hc + BL, hc * P : (hc + 1) * P],
                        st4_sb[32 * hc : 32 * hc + BL, j * MT : (j + 1) * MT],
                        start=False,
                        stop=True,
                        tile_position=(32 * hc, 0),
                    )
                for hc in range(HC):
                    if hc % 2 == 0:
                        nc.scalar.activation(h1_sb[:, hc, :], h1ps[hc], AF.Relu)
                    else:
                        nc.vector.tensor_scalar_max(h1_sb[:, hc, :], h1ps[hc], 0.0)

                # previous tile's e transpose + one-hot scaling (PE+DVE,
                # overlaps this tile's h2)
                if prev is not None:
                    pending.append((prev[0], prev[1], emit_e(prev[0], prev[1], prev[2])))
                    prev = None

                # h2
                h2_sb = wpool.tile([P, KC, MT], FP8, tag="h2", bufs=2)
                for hc in range(HC):
                    h2p = ppool.tile([P, MT], FP32, tag="hp", bufs=4)
                    for kc in range(0, KC, 2):
                        nc.tensor.matmul(
                            h2p,
                            w2_sb[:, kc : kc + 2, hc * P : (hc + 1) * P],
                            h1_sb[:, kc : kc + 2, :],
                            start=(kc == 0),
                            stop=(kc == KC - 2),
                            perf_mode=mybir.MatmulPerfMode.DoubleRow,
                        )
                    if hc % 2 == 0:
                        nc.scalar.activation(
                            h2_sb[:, hc, :], h2p, AF.Relu, bias=b2_sb[:, hc : hc + 1]
                        )
                    else:
                        nc.vector.tensor_scalar(
                            h2_sb[:, hc, :],
                            h2p,
                            b2_sb[:, hc : hc + 1],
                            0.0,
                            op0=ALU.add,
                            op1=ALU.max,
                        )

                # pooling matmuls run two tiles behind (4 col-tiled,
                # concurrent): their ps tile is guaranteed resident
                if len(pending) >= 4:
                    pj, p_ps, p_stmw = pending.pop(0)
                    emit_pool(pj, p_ps, p_stmw)

                # logits -> e = exp(logits)   (b3 dropped: cancels in softmax)
                e_row = wpool.tile([1, MT], FP32, tag="erow", bufs=2)
                lp = ppool.tile([1, MT], FP32, tag="lp", bufs=1)
                for kc in range(0, KC, 2):
                    nc.tensor.matmul(
                        lp,
                        w3_sb[:, kc : kc + 2, 0:1],
                        h2_sb[:, kc : kc + 2, :],
                        start=(kc == 0),
                        stop=(kc == KC - 2),
                        perf_mode=mybir.MatmulPerfMode.DoubleRow,
                    )
                nc.scalar.activation(e_row, lp, AF.Exp)

                prev = (j, ps_sb, e_row)

            # drain: last tile's e + remaining pools
            pending.append((prev[0], prev[1], emit_e(prev[0], prev[1], prev[2])))
            for pj, p_ps, p_stmw in pending:
                emit_pool(pj, p_ps, p_stmw)

            # ---------------- local finalize (no collectives) ----------------
            # fold the four pool regions with a select matmul
            poolc_sb = wpool.tile([P, H], BF16, tag="fin_poolc", bufs=1)
            nc.vector.tensor_copy(poolc_sb, pool_psum)
            pool8 = ppool.tile([P, H], FP32, tag="hp", bufs=4)
            nc.tensor.matmul(
                pool8[0:BL, :], sel_sb, poolc_sb, start=True, stop=True
            )
            # den: [1, NSUB*BL] -> [1, BL] (sum subtiles) -> [BL, 1]
            denr_sb = wpool.tile([1, NSUB * BL], FP32, tag="fin_denr", bufs=1)
            nc.vector.tensor_copy(denr_sb, den_psum)
            den1_sb = wpool.tile([1, BL], FP32, tag="fin_den1", bufs=1)
            nc.vector.tensor_add(
                den1_sb, denr_sb[0:1, 0:BL], denr_sb[0:1, BL : 2 * BL]
            )
            nc.vector.tensor_add(
                den1_sb, den1_sb, denr_sb[0:1, 2 * BL : 3 * BL]
            )
            nc.vector.tensor_add(
                den1_sb, den1_sb, denr_sb[0:1, 3 * BL : 4 * BL]
            )
            denTp = ppool.tile([BL, 1], FP32, tag="eT", bufs=1)
            nc.tensor.transpose(denTp, den1_sb, identf[0:1, 0:1])
            rec = wpool.tile([BL, 1], FP32, tag="fin_rec", bufs=1)
            nc.vector.reciprocal(rec, denTp)
            pooled = wpool.tile([BL, H], FP32, tag="fin_pool", bufs=1)
            nc.vector.tensor_scalar_mul(pooled, pool8[0:BL, :], rec[:, 0:1])

            # final_fc on this core's BL segment rows
            ptp = ppool.tile([P, KC * BL], FP32, tag="eT", bufs=1)
            for kc in range(KC):
                nc.tensor.transpose(
                    ptp[:, kc * BL : (kc + 1) * BL],
                    pooled[:, kc * P : (kc + 1) * P],
                    identbr,
                )
            pooledT = wpool.tile([P, KC * BL], BF16, tag="fin_poolT", bufs=1)
            nc.vector.tensor_copy(pooledT, ptp)

            hf_sb = wpool.tile([P, HC * BL], BF16, tag="fin_hf", bufs=1)
            for hc in range(HC):
                hfp = ppool.tile([P, BL], FP32, tag="hp", bufs=4)
                for kc in range(KC):
                    nc.tensor.matmul(
                        hfp,
                        wf1_sb[:, kc, hc * P : (hc + 1) * P],
                        pooledT[:, kc * BL : (kc + 1) * BL],
                        start=(kc == 0),
                        stop=(kc == KC - 1),
                    )
                nc.scalar.activation(
                    hf_sb[:, hc * BL : (hc + 1) * BL],
                    hfp,
                    AF.Relu,
                    bias=bf1_sb[:, hc : hc + 1],
                )
            op = ppool.tile([2, BL], FP32, tag="lp", bufs=1)
            for hc in range(HC):
                nc.tensor.matmul(
                    op,
                    wf2_sb[:, hc, :],
                    hf_sb[:, hc * BL : (hc + 1) * BL],
                    start=(hc == 0),
                    stop=(hc == HC - 1),
                )
            o_sb = wpool.tile([2, BL], FP32, tag="fin_o", bufs=1)
            nc.vector.tensor_scalar_add(o_sb, op, bf2_sb[:, 0:1])
            nc.sync.dma_start(outT, o_sb)

    nc.compile()
    return nc


def prep_in_maps(inputs, tloc=TLOC, ncores=NCORES):
    """Shard the full inputs into per-core input maps (host-side prep only:
    segment-aligned slicing, layout transposes, dtype casts, one-hot
    materialization, zero padding)."""
    bf = ml_dtypes.bfloat16
    f8 = ml_dtypes.float8_e4m3
    nt = tloc // MT
    ps = np.ascontiguousarray(np.asarray(inputs["ps_data"], np.float32))
    sid = np.asarray(inputs["segment_ids"], np.int64)
    io_flat = np.asarray(inputs["io_embed"], np.float32).reshape(B, -1)
    ttot = ps.shape[0]
    assert sid.shape[0] == ttot

    # segment-aligned split: core c owns all tokens of segments [8c, 8c+8)
    counts = np.bincount(sid, minlength=B)
    starts = np.zeros(B + 1, np.int64)
    np.cumsum(counts, out=starts[1:])

    W1 = np.asarray(inputs["W1"], np.float32)
    sel_host = np.zeros((P, BL), bf)
    rep4_host = np.zeros((BL, P), bf)
    for s in range(NSUB):
        for i in range(BL):
            sel_host[32 * s + i, i] = 1
            rep4_host[i, 32 * s + i] = 1

    shared = {
        "w1b": W1[X:].reshape(P, NKB, H).astype(bf),
        "b1": np.asarray(inputs["b1"], np.float32).reshape(1, H).astype(bf),
        "w1a": np.ascontiguousarray(
            W1[:X].reshape(KC, P, H).transpose(1, 0, 2)
        ).astype(f8),
        "w2": np.ascontiguousarray(
            np.asarray(inputs["W2"], np.float32).reshape(KC, P, H).transpose(1, 0, 2)
        ).astype(f8),
        "b2": np.ascontiguousarray(
            np.asarray(inputs["b2"], np.float32).reshape(HC, P).T
        ),
        "w3": np.ascontiguousarray(
            np.asarray(inputs["W3"], np.float32).reshape(KC, P, 1).transpose(1, 0, 2)
        ).astype(f8),
        "sel": sel_host,
        "rep4": rep4_host,
        "wf1": np.ascontiguousarray(
            np.asarray(inputs["Wf1"], np.float32).reshape(KC, P, H).transpose(1, 0, 2)
        ).astype(bf),
        "bf1": np.ascontiguousarray(
            np.asarray(inputs["bf1"], np.float32).reshape(HC, P).T
        ),
        "wf2": np.ascontiguousarray(
            np.asarray(inputs["Wf2"], np.float32).reshape(KC, P, 2).transpose(1, 0, 2)
        ).astype(bf),
        "bf2": np.asarray(inputs["bf2"], np.float32).reshape(2, 1).copy(),
    }
    in_maps = []
    for c in range(ncores):
        lo, hi = starts[c * BL], starts[(c + 1) * BL]
        cnt = int(hi - lo)
        assert cnt <= tloc, f"core {c} owns {cnt} tokens > tloc={tloc}"
        psc = np.zeros((tloc, X), np.float32)
        psc[:cnt] = ps[lo:hi]
        sidl = sid[lo:hi] - c * BL  # local segment ids 0..BL-1
        # feature-major fp8 for the MLP path: [nt, P, KC, MT],
        # [j, p, kc, m] = psc[j*MT + m, kc*P + p]
        psT_c = np.ascontiguousarray(
            psc.reshape(nt, MT, KC, P).transpose(0, 3, 2, 1)
        ).astype(f8)
        # token-major bf16 for the pool path with the local one-hot in the
        # last BL columns: [nt, P, NSUB, X+BL],
        # [j, p, s, x] = aug[j*MT + s*P + p, x]
        oh8 = np.zeros((tloc, BL), np.float32)
        oh8[np.arange(cnt), sidl] = 1
        aug = np.concatenate([psc, oh8], axis=1)
        ps_c = np.ascontiguousarray(
            aug.reshape(nt, NSUB, P, X + BL).transpose(0, 2, 1, 3)
        ).astype(bf)
        # st4: local one-hot transposed, replicated in the 4 row groups
        st4_c = np.zeros((P, tloc), bf)
        oh8T = oh8.astype(bf).T
        for g in range(HC):
            st4_c[32 * g : 32 * g + BL, :] = oh8T
        ioT_c = np.ascontiguousarray(
            io_flat[c * BL : (c + 1) * BL].T
        ).reshape(P, NKB, BL).astype(bf)
        in_maps.append(
            {
                "psT": psT_c,
                "ps": ps_c,
                "st4": st4_c,
                "ioT": ioT_c,
                **shared,
            }
        )
    return in_maps


_NC_CACHE = {}


def _get_nc(tloc=TLOC):
    if tloc not in _NC_CACHE:
        _NC_CACHE[tloc] = build(tloc)
    return _NC_CACHE[tloc]


def run(inputs, trace=False):
    sid = np.asarray(inputs["segment_ids"], np.int64)
    counts = np.bincount(sid, minlength=B)
    mx = int(
        max(counts[c * BL : (c + 1) * BL].sum() for c in range(NCORES))
    )
    tloc = max(TLOC, ((mx + MT - 1) // MT) * MT)
    nc = _get_nc(tloc)
    in_maps = prep_in_maps(inputs, tloc=tloc)
    res = run_bass_kernel_spmd(nc, in_maps, core_ids=list(range(NCORES)), trace=trace)
    out = np.concatenate(
        [res.results[c]["outT"].T for c in range(NCORES)], axis=0
    ).astype(np.float32)
    return np.ascontiguousarray(out), res


def kernel(**inputs):
    out, _ = run(inputs)
    return out


# revision 46
# speedup vs baseline: 1.0118x; 1.0118x over previous
"""Trainium2 Bass kernel for LGRL classifier decoder (segment softmax-pool MLP).

Math (reference):
    extra = io_embed.reshape(B, Y)[segment_ids]                # (T, Y)
    h1 = relu([ps_data, extra] @ W1 + b1)
    h2 = relu(h1 @ W2 + b2)
    logits = (h2 @ W3 + b3)[:, 0]
    w = segment_softmax(logits)
    pooled = segment_sum(w * ps_data)                          # (B, X)
    out = relu(pooled @ Wf1 + bf1) @ Wf2 + bf2                 # (B, 2)

Key transformations:
  * segment-aligned sharding: segment_ids are sorted, so the host assigns
    core c ALL tokens of segments [8c, 8c+8), padded to a fixed tloc with
    dummy tokens whose one-hot columns are zero (exact no-op in every
    reduction).  All segment reductions become core-local: NO collectives.
  * [ps, extra] @ W1 = ps @ W1a + onehot8(seg) @ (io8 @ W1b + b1): the
    extra-part matmul collapses to a (8, Y) @ (Y, H) on-device precompute
    plus a rank-8 one-hot matmul; with K=8 the four hc-chunk matmuls run
    CONCURRENTLY in four 32-row groups of the PE array (row tiling).
  * pooling scales the 8-col one-hot by e and runs the four subtile
    matmuls concurrently in four 32-partition output groups (col tiling);
    a final select-matrix matmul folds the four groups.
  * per-segment max subtraction in the softmax is dropped (shift
    invariant, logits are O(0.1)); b3 dropped for the same reason.
  * the host ships ps twice: token-major bf16 (pool path) and
    feature-major fp8 (MLP path), pre-arranged to the exact SBUF layout:
    no PE transposes, no dtype-converting DMAs.
  * h1/h2/logits matmuls run fp8 DoubleRow (measured ~216ns per
    FD=512 matmul with LDWEIGHTS fully hidden), fp32 PSUM accumulation.
"""

import numpy as np
import ml_dtypes

import concourse.bass as bass
import concourse.mybir as mybir
import concourse.tile as tile
from concourse import bacc
from concourse.bass_utils import run_bass_kernel_spmd
from concourse.masks import make_identity

B = 64
T = 65536
X = 512
KIO = 5
Y = X * KIO          # 2560
H = 512
NCORES = 8
P = 128
FP32 = mybir.dt.float32
BF16 = mybir.dt.bfloat16
FP8 = mybir.dt.float8e4
AF = mybir.ActivationFunctionType
ALU = mybir.AluOpType

KC = X // P          # 4 contraction chunks for 512-dims
HC = H // P          # 4 output chunks for 512-dims
NKB = Y // P         # 20 contraction chunks of W1b
MT = 512             # tokens per MLP tile
NSUB = MT // P       # 128-token subtiles per MLP tile
BL = B // NCORES     # segments owned per core (local)
TLOC = 8704          # per-core padded token count (8192 + 512 slack)


def build(tloc=TLOC):
    """Build + compile the SPMD kernel for per-core token count `tloc`."""
    nt = tloc // MT
    nc = bacc.Bacc(
        "TRN2", target_bir_lowering=False, debug=False, num_devices=NCORES
    )

    psT = nc.dram_tensor("psT", [nt, P, KC, MT], FP8, kind="ExternalInput").ap()
    # ps carries the 8-col local one-hot in columns X:X+BL (one DMA/tile)
    ps = nc.dram_tensor("ps", [nt, P, NSUB, X + BL], BF16, kind="ExternalInput").ap()
    st4 = nc.dram_tensor("st4", [P, tloc], BF16, kind="ExternalInput").ap()
    ioT = nc.dram_tensor("ioT", [P, NKB, BL], BF16, kind="ExternalInput").ap()
    w1b = nc.dram_tensor("w1b", [P, NKB, H], BF16, kind="ExternalInput").ap()
    b1 = nc.dram_tensor("b1", [1, H], BF16, kind="ExternalInput").ap()
    w1a = nc.dram_tensor("w1a", [P, KC, H], FP8, kind="ExternalInput").ap()
    w2 = nc.dram_tensor("w2", [P, KC, H], FP8, kind="ExternalInput").ap()
    b2 = nc.dram_tensor("b2", [P, HC], FP32, kind="ExternalInput").ap()
    w3 = nc.dram_tensor("w3", [P, KC, 1], FP8, kind="ExternalInput").ap()
    sel = nc.dram_tensor("sel", [P, BL], BF16, kind="ExternalInput").ap()
    rep4 = nc.dram_tensor("rep4", [BL, P], BF16, kind="ExternalInput").ap()
    wf1 = nc.dram_tensor("wf1", [P, KC, H], BF16, kind="ExternalInput").ap()
    bf1_t = nc.dram_tensor("bf1", [P, HC], FP32, kind="ExternalInput").ap()
    wf2 = nc.dram_tensor("wf2", [P, KC, 2], BF16, kind="ExternalInput").ap()
    bf2_t = nc.dram_tensor("bf2", [2, 1], FP32, kind="ExternalInput").ap()
    outT = nc.dram_tensor("outT", [2, BL], FP32, kind="ExternalOutput").ap()

    with tile.TileContext(nc) as tc:
        with (
            tc.tile_pool(name="const", bufs=1) as cpool,
            tc.tile_pool(name="work", bufs=2) as wpool,
            tc.tile_pool(name="psum", bufs=1, space="PSUM") as ppool,
        ):
            # ---------------- constants ----------------
            identf = cpool.tile([1, 1], FP32)
            nc.gpsimd.memset(identf, 1.0)
            ones_b = cpool.tile([1, BL], BF16)
            nc.gpsimd.memset(ones_b, 1.0)
            ones_col = cpool.tile([P, 1], BF16)
            nc.gpsimd.memset(ones_col, 1.0)
            identbr = cpool.tile([BL, BL], FP32)
            make_identity(nc, identbr)

            # DMA issue order is tuned so the first ~10us keeps every queue
            # streaming what the PE consumes first: ps0 (pool of tile 0) on
            # sync ahead of w1b's sync share; w1b split scalar/sync feeding
            # the seg-block matmuls; st4's first chunk on gpsimd.
            def _tile_dma(j, ps_eng=None):
                psT_sb = wpool.tile(
                    [P, KC, MT], FP8, tag="psT", bufs=6, name=f"psT_{j}"
                )
                (nc.scalar if j % 2 == 0 else nc.sync).dma_start(psT_sb, psT[j])
                ps_sb = wpool.tile(
                    [P, NSUB, X + BL], BF16, tag="ps", bufs=7, name=f"ps_{j}"
                )
                (ps_eng or nc.gpsimd).dma_start(ps_sb, ps[j])
                return psT_sb, ps_sb

            ps0_sb = wpool.tile(
                [P, NSUB, X + BL], BF16, tag="ps", bufs=7, name="ps_0"
            )
            nc.gpsimd.dma_start(ps0_sb, ps[0])

            ioT_sb = cpool.tile([P, NKB, BL], BF16)
            nc.scalar.dma_start(ioT_sb, ioT)
            rep4_sb = cpool.tile([BL, P], BF16)
            nc.scalar.dma_start(rep4_sb, rep4)
            w1b_sb = wpool.tile([P, NKB, H], BF16, tag="w1b", bufs=1)
            Q = NKB // 4
            for q in range(4):
                eng = nc.scalar if q < 2 else nc.sync
                eng.dma_start(
                    w1b_sb[:, q * Q : (q + 1) * Q, :], w1b[:, q * Q : (q + 1) * Q, :]
                )
            b1_sb = cpool.tile([1, H], BF16)
            nc.scalar.dma_start(b1_sb, b1)
            w1a_sb = cpool.tile([P, KC, H], FP8)
            nc.scalar.dma_start(w1a_sb, w1a)

            psT0_sb = wpool.tile([P, KC, MT], FP8, tag="psT", bufs=6, name="psT_0")
            nc.scalar.dma_start(psT0_sb, psT[0])
            pre = [(psT0_sb, ps0_sb)]

            st4_sb = cpool.tile([P, tloc], BF16)
            CH = tloc // 8
            nc.sync.dma_start(st4_sb[:, 0:CH], st4[:, 0:CH])
            NPRE = min(3, nt)
            for j in range(1, NPRE):
                pre.append(_tile_dma(j))
            nc.sync.dma_start(st4_sb[:, CH : 2 * CH], st4[:, CH : 2 * CH])

            w2_sb = cpool.tile([P, KC, H], FP8)
            nc.scalar.dma_start(w2_sb, w2)
            # 16-wide w3 tile keeps the DoubleRow pair step a multiple of 16B
            w3_sb = cpool.tile([P, KC, 16], FP8)
            nc.scalar.dma_start(w3_sb[:, :, 0:1], w3)
            b2_sb = cpool.tile([P, HC], FP32)
            nc.scalar.dma_start(b2_sb, b2)
            sel_sb = cpool.tile([P, BL], BF16)
            nc.scalar.dma_start(sel_sb, sel)
            # late st4 chunks ride the gpsimd queue behind the early ps
            # tiles so they never delay psT on the HWDGE queues
            for q in range(2, 4):
                nc.gpsimd.dma_start(
                    st4_sb[:, q * CH : (q + 1) * CH], st4[:, q * CH : (q + 1) * CH]
                )
            # final-fc weights are only needed at the tail; allocate now,
            # DMA later (emitted just before the finalize section)
            wf1_sb = cpool.tile([P, KC, H], BF16)
            bf1_sb = cpool.tile([P, HC], FP32)
            wf2_sb = cpool.tile([P, KC, 2], BF16)
            bf2_sb = cpool.tile([2, 1], FP32)

            # ---------------- seg8 = io8 @ W1b + b1  (BL, H) ----------------
            seg_psum = ppool.tile([P, H], FP32, tag="poolacc", bufs=1)
            for kb in range(NKB):
                nc.tensor.matmul(
                    seg_psum[0:BL, :],
                    ioT_sb[:, kb, :],
                    w1b_sb[:, kb, :],
                    start=(kb == 0),
                    stop=False,
                )
            nc.tensor.matmul(
                seg_psum[0:BL, :], ones_b, b1_sb, start=False, stop=True
            )
            seg_sb = cpool.tile([BL, H], BF16)
            nc.vector.tensor_copy(seg_sb, seg_psum[0:BL, :])
            # replicate seg8 into the four 32-row groups via one select
            # matmul (avoids queue-blocking SBUF->SBUF DMAs)
            NR = 3 * 32 + BL  # 104 rows cover all four groups
            seg_rep = ppool.tile([P, H], FP32, tag="lp", bufs=1)
            nc.tensor.matmul(
                seg_rep[0:NR, :], rep4_sb[:, 0:NR], seg_sb, start=True, stop=True
            )
            seg_dup = cpool.tile([P, H], BF16)
            nc.vector.tensor_copy(seg_dup[0:NR, :], seg_rep[0:NR, :])

            # ---------------- main loop over MLP tiles ----------------
            # pool regions: subtile s accumulates into partitions
            # [32s, 32s+BL); a final sel-matmul folds the four regions.
            pool_psum = ppool.tile([P, H], FP32, tag="poolacc", bufs=1)
            den_psum = ppool.tile([1, NSUB * BL], FP32, tag="den", bufs=1)
            prev = None  # (j, ps_sb, e_row) of previous tile
            pending = []  # [(j, ps_sb, stmw)] awaiting pool matmuls

            def emit_e(pj, p_ps, p_erow):
                eTp = ppool.tile([P, NSUB], FP32, tag="eT", bufs=1)
                for s in range(NSUB):
                    nc.tensor.transpose(
                        eTp[:, s : s + 1],
                        p_erow[0:1, s * P : (s + 1) * P],
                        identf[0:1, 0:1],
                    )
                e_col = wpool.tile([P, NSUB], FP32, tag="ecol", bufs=2)
                nc.vector.tensor_copy(e_col, eTp)
                stmw = wpool.tile([P, NSUB, BL], BF16, tag="stmw", bufs=5)
                for s in range(NSUB):
                    nc.vector.tensor_scalar_mul(
                        stmw[:, s, :],
                        p_ps[:, s, X : X + BL],
                        e_col[:, s : s + 1],
                    )
                return stmw

            def emit_pool(pj, p_ps, stmw):
                first = pj == 0
                last = pj == nt - 1
                for s in range(NSUB):
                    nc.tensor.matmul(
                        pool_psum[32 * s : 32 * s + BL, :],
                        stmw[:, s, :],
                        p_ps[:, s, 0:X],
                        start=first,
                        stop=last,
                        tile_position=(0, 32 * s),
                        skip_group_check=True,
                    )
                nc.tensor.matmul(
                    den_psum[0:1, :],
                    ones_col,
                    stmw[:, :, :],
                    start=first,
                    stop=last,
                )

            for j in range(nt):
                psT_sb, ps_sb = pre[j] if j < NPRE else _tile_dma(j)
                if j == 4:
                    for q in range(4, 6):
                        nc.gpsimd.dma_start(
                            st4_sb[:, q * CH : (q + 1) * CH],
                            st4[:, q * CH : (q + 1) * CH],
                        )
                if j == 8:
                    for q in range(6, 8):
                        nc.gpsimd.dma_start(
                            st4_sb[:, q * CH : (q + 1) * CH],
                            st4[:, q * CH : (q + 1) * CH],
                        )
                if j == 10:
                    # final-fc weights: issued mid-loop on gpsimd so they
                    # land well before the tail without delaying any psT
                    nc.gpsimd.dma_start(wf1_sb, wf1)
                    nc.gpsimd.dma_start(bf1_sb, bf1_t)
                    nc.gpsimd.dma_start(wf2_sb, wf2)
                    nc.gpsimd.dma_start(bf2_sb, bf2_t)

                # h1 = relu(psT-major matmuls + seg8 one-hot broadcast);
                # the four K=8 seg matmuls run concurrently in four 32-row
                # groups of the PE array.
                h1_sb = wpool.tile([P, KC, MT], FP8, tag="h1", bufs=2)
                h1ps = []
                for hc in range(HC):
                    h1p = ppool.tile([P, MT], FP32, tag="hp", bufs=4)
                    for kc in range(0, KC, 2):
                        nc.tensor.matmul(
                            h1p,
                            w1a_sb[:, kc : kc + 2, hc * P : (hc + 1) * P],
                            psT_sb[:, kc : kc + 2, :],
                            start=(kc == 0),
                            stop=False,
                            perf_mode=mybir.MatmulPerfMode.DoubleRow,
                        )
                    h1ps.append(h1p)
                for hc in range(HC):
                    nc.tensor.matmul(
                        h1ps[hc],
                        seg_dup[32 * hc : 32 * hc + BL, hc * P : (hc + 1) * P],
                        st4_sb[32 * hc : 32 * hc + BL, j * MT : (j + 1) * MT],
                        start=False,
                        stop=True,
                        tile_position=(32 * hc, 0),
                    )
                for hc in range(HC):
                    if hc % 2 == 0:
                        nc.scalar.activation(h1_sb[:, hc, :], h1ps[hc], AF.Relu)
                    else:
                        nc.vector.tensor_scalar_max(h1_sb[:, hc, :], h1ps[hc], 0.0)

                # previous tile's e transpose + one-hot scaling (PE+DVE,
                # overlaps this tile's h2)
                if prev is not None:
                    pending.append((prev[0], prev[1], emit_e(prev[0], prev[1], prev[2])))
                    prev = None

                # h2
                h2_sb = wpool.tile([P, KC, MT], FP8, tag="h2", bufs=2)
                for hc in range(HC):
                    h2p = ppool.tile([P, MT], FP32, tag="hp", bufs=4)
                    for kc in range(0, KC, 2):
                        nc.tensor.matmul(
                            h2p,
                            w2_sb[:, kc : kc + 2, hc * P : (hc + 1) * P],
                            h1_sb[:, kc : kc + 2, :],
                            start=(kc == 0),
                            stop=(kc == KC - 2),
                            perf_mode=mybir.MatmulPerfMode.DoubleRow,
                        )
                    if hc % 2 == 0:
                        nc.scalar.activation(
                            h2_sb[:, hc, :], h2p, AF.Relu, bias=b2_sb[:, hc : hc + 1]
                        )
                    else:
                        nc.vector.tensor_scalar(
                            h2_sb[:, hc, :],
                            h2p,
                            b2_sb[:, hc : hc + 1],
                            0.0,
                            op0=ALU.add,
                            op1=ALU.max,
                        )

                # pooling matmuls run two tiles behind (4 col-tiled,
                # concurrent): their ps tile is guaranteed resident
                if len(pending) >= 4:
                    pj, p_ps, p_stmw = pending.pop(0)
                    emit_pool(pj, p_ps, p_stmw)

                # logits -> e = exp(logits)   (b3 dropped: cancels in softmax)
                e_row = wpool.tile([1, MT], FP32, tag="erow", bufs=2)
                lp = ppool.tile([1, MT], FP32, tag="lp", bufs=1)
                for kc in range(0, KC, 2):
                    nc.tensor.matmul(
                        lp,
                        w3_sb[:, kc : kc + 2, 0:1],
                        h2_sb[:, kc : kc + 2, :],
                        start=(kc == 0),
                        stop=(kc == KC - 2),
                        perf_mode=mybir.MatmulPerfMode.DoubleRow,
                    )
                nc.scalar.activation(e_row, lp, AF.Exp)

                prev = (j, ps_sb, e_row)

            # drain: last tile's e + remaining pools
            pending.append((prev[0], prev[1], emit_e(prev[0], prev[1], prev[2])))
            for pj, p_ps, p_stmw in pending:
                emit_pool(pj, p_ps, p_stmw)

            # ---------------- local finalize (no collectives) ----------------
            # fold the four pool regions with a select matmul
            poolc_sb = wpool.tile([P, H], BF16, tag="fin_poolc", bufs=1)
            nc.vector.tensor_copy(poolc_sb, pool_psum)
            pool8 = ppool.tile([P, H], FP32, tag="hp", bufs=4)
            nc.tensor.matmul(
                pool8[0:BL, :], sel_sb, poolc_sb, start=True, stop=True
            )
            # den: [1, NSUB*BL] -> [1, BL] (sum subtiles) -> [BL, 1]
            denr_sb = wpool.tile([1, NSUB * BL], FP32, tag="fin_denr", bufs=1)
            nc.vector.tensor_copy(denr_sb, den_psum)
            den1_sb = wpool.tile([1, BL], FP32, tag="fin_den1", bufs=1)
            nc.vector.tensor_add(
                den1_sb, denr_sb[0:1, 0:BL], denr_sb[0:1, BL : 2 * BL]
            )
            nc.vector.tensor_add(
                den1_sb, den1_sb, denr_sb[0:1, 2 * BL : 3 * BL]
            )
            nc.vector.tensor_add(
                den1_sb, den1_sb, denr_sb[0:1, 3 * BL : 4 * BL]
            )
            denTp = ppool.tile([BL, 1], FP32, tag="eT", bufs=1)
            nc.tensor.transpose(denTp, den1_sb, identf[0:1, 0:1])
            rec = wpool.tile([BL, 1], FP32, tag="fin_rec", bufs=1)
            nc.vector.reciprocal(rec, denTp)
            pooled = wpool.tile([BL, H], FP32, tag="fin_pool", bufs=1)
            nc.vector.tensor_scalar_mul(pooled, pool8[0:BL, :], rec[:, 0:1])

            # final_fc on this core's BL segment rows
            ptp = ppool.tile([P, KC * BL], FP32, tag="eT", bufs=1)
            for kc in range(KC):
                nc.tensor.transpose(
                    ptp[:, kc * BL : (kc + 1) * BL],
                    pooled[:, kc * P : (kc + 1) * P],
                    identbr,
                )
            pooledT = wpool.tile([P, KC * BL], BF16, tag="fin_poolT", bufs=1)
            nc.vector.tensor_copy(pooledT, ptp)

            hf_sb = wpool.tile([P, HC * BL], BF16, tag="fin_hf", bufs=1)
            for hc in range(HC):
                hfp = ppool.tile([P, BL], FP32, tag="hp", bufs=4)
                for kc in range(KC):
                    nc.tensor.matmul(
                        hfp,
                        wf1_sb[:, kc, hc * P : (hc + 1) * P],
                        pooledT[:, kc * BL : (kc + 1) * BL],
                        start=(kc == 0),
                        stop=(kc == KC - 1),
                    )
                nc.scalar.activation(
                    hf_sb[:, hc * BL : (hc + 1) * BL],
                    hfp,
                    AF.Relu,
                    bias=bf1_sb[:, hc : hc + 1],
                )
            op = ppool.tile([2, BL], FP32, tag="lp", bufs=1)
            for hc in range(HC):
                nc.tensor.matmul(
                    op,
                    wf2_sb[:, hc, :],
                    hf_sb[:, hc * BL : (hc + 1) * BL],
                    start=(hc == 0),
                    stop=(hc == HC - 1),
                )
            o_sb = wpool.tile([2, BL], FP32, tag="fin_o", bufs=1)
            nc.vector.tensor_scalar_add(o_sb, op, bf2_sb[:, 0:1])
            nc.sync.dma_start(outT, o_sb)

    nc.compile()
    return nc


def prep_in_maps(inputs, tloc=TLOC, ncores=NCORES):
    """Shard the full inputs into per-core input maps (host-side prep only:
    segment-aligned slicing, layout transposes, dtype casts, one-hot
    materialization, zero padding)."""
    bf = ml_dtypes.bfloat16
    f8 = ml_dtypes.float8_e4m3
    nt = tloc // MT
    ps = np.ascontiguousarray(np.asarray(inputs["ps_data"], np.float32))
    sid = np.asarray(inputs["segment_ids"], np.int64)
    io_flat = np.asarray(inputs["io_embed"], np.float32).reshape(B, -1)
    ttot = ps.shape[0]
    assert sid.shape[0] == ttot

    # segment-aligned split: core c owns all tokens of segments [8c, 8c+8)
    counts = np.bincount(sid, minlength=B)
    starts = np.zeros(B + 1, np.int64)
    np.cumsum(counts, out=starts[1:])

    W1 = np.asarray(inputs["W1"], np.float32)
    sel_host = np.zeros((P, BL), bf)
    rep4_host = np.zeros((BL, P), bf)
    for s in range(NSUB):
        for i in range(BL):
            sel_host[32 * s + i, i] = 1
            rep4_host[i, 32 * s + i] = 1

    shared = {
        "w1b": W1[X:].reshape(P, NKB, H).astype(bf),
        "b1": np.asarray(inputs["b1"], np.float32).reshape(1, H).astype(bf),
        "w1a": np.ascontiguousarray(
            W1[:X].reshape(KC, P, H).transpose(1, 0, 2)
        ).astype(f8),
        "w2": np.ascontiguousarray(
            np.asarray(inputs["W2"], np.float32).reshape(KC, P, H).transpose(1, 0, 2)
        ).astype(f8),
        "b2": np.ascontiguousarray(
            np.asarray(inputs["b2"], np.float32).reshape(HC, P).T
        ),
        "w3": np.ascontiguousarray(
            np.asarray(inputs["W3"], np.float32).reshape(KC, P, 1).transpose(1, 0, 2)
        ).astype(f8),
        "sel": sel_host,
        "rep4": rep4_host,
        "wf1": np.ascontiguousarray(
            np.asarray(inputs["Wf1"], np.float32).reshape(KC, P, H).transpose(1, 0, 2)
        ).astype(bf),
        "bf1": np.ascontiguousarray(
            np.asarray(inputs["bf1"], np.float32).reshape(HC, P).T
        ),
        "wf2": np.ascontiguousarray(
            np.asarray(inputs["Wf2"], np.float32).reshape(KC, P, 2).transpose(1, 0, 2)
        ).astype(bf),
        "bf2": np.asarray(inputs["bf2"], np.float32).reshape(2, 1).copy(),
    }
    in_maps = []
    for c in range(ncores):
        lo, hi = starts[c * BL], starts[(c + 1) * BL]
        cnt = int(hi - lo)
        assert cnt <= tloc, f"core {c} owns {cnt} tokens > tloc={tloc}"
        psc = np.zeros((tloc, X), np.float32)
        psc[:cnt] = ps[lo:hi]
        sidl = sid[lo:hi] - c * BL  # local segment ids 0..BL-1
        # feature-major fp8 for the MLP path: [nt, P, KC, MT],
        # [j, p, kc, m] = psc[j*MT + m, kc*P + p]
        psT_c = np.ascontiguousarray(
            psc.reshape(nt, MT, KC, P).transpose(0, 3, 2, 1)
        ).astype(f8)
        # token-major bf16 for the pool path with the local one-hot in the
        # last BL columns: [nt, P, NSUB, X+BL],
        # [j, p, s, x] = aug[j*MT + s*P + p, x]
        oh8 = np.zeros((tloc, BL), np.float32)
        oh8[np.arange(cnt), sidl] = 1
        aug = np.concatenate([psc, oh8], axis=1)
        ps_c = np.ascontiguousarray(
            aug.reshape(nt, NSUB, P, X + BL).transpose(0, 2, 1, 3)
        ).astype(bf)
        # st4: local one-hot transposed, replicated in the 4 row groups
        st4_c = np.zeros((P, tloc), bf)
        oh8T = oh8.astype(bf).T
        for g in range(HC):
            st4_c[32 * g : 32 * g + BL, :] = oh8T
        ioT_c = np.ascontiguousarray(
            io_flat[c * BL : (c + 1) * BL].T
        ).reshape(P, NKB, BL).astype(bf)
        in_maps.append(
            {
                "psT": psT_c,
                "ps": ps_c,
                "st4": st4_c,
                "ioT": ioT_c,
                **shared,
            }
        )
    return in_maps


_NC_CACHE = {}


def _get_nc(tloc=TLOC):
    if tloc not in _NC_CACHE:
        _NC_CACHE[tloc] = build(tloc)
    return _NC_CACHE[tloc]


def run(inputs, trace=False):
    sid = np.asarray(inputs["segment_ids"], np.int64)
    counts = np.bincount(sid, minlength=B)
    mx = int(
        max(counts[c * BL : (c + 1) * BL].sum() for c in range(NCORES))
    )
    tloc = max(TLOC, ((mx + MT - 1) // MT) * MT)
    nc = _get_nc(tloc)
    in_maps = prep_in_maps(inputs, tloc=tloc)
    res = run_bass_kernel_spmd(nc, in_maps, core_ids=list(range(NCORES)), trace=trace)
    out = np.concatenate(
        [res.results[c]["outT"].T for c in range(NCORES)], axis=0
    ).astype(np.float32)
    return np.ascontiguousarray(out), res


def kernel(**inputs):
    out, _ = run(inputs)
    return out


# revision 47
# speedup vs baseline: 1.0350x; 1.0230x over previous
"""Trainium2 Bass kernel for LGRL classifier decoder (segment softmax-pool MLP).

Math (reference):
    extra = io_embed.reshape(B, Y)[segment_ids]                # (T, Y)
    h1 = relu([ps_data, extra] @ W1 + b1)
    h2 = relu(h1 @ W2 + b2)
    logits = (h2 @ W3 + b3)[:, 0]
    w = segment_softmax(logits)
    pooled = segment_sum(w * ps_data)                          # (B, X)
    out = relu(pooled @ Wf1 + bf1) @ Wf2 + bf2                 # (B, 2)

Key transformations:
  * segment-aligned sharding: segment_ids are sorted, so the host assigns
    core c ALL tokens of segments [8c, 8c+8), padded to a fixed tloc with
    dummy tokens whose one-hot columns are zero (exact no-op in every
    reduction).  All segment reductions become core-local: NO collectives.
  * [ps, extra] @ W1 = ps @ W1a + onehot8(seg) @ (io8 @ W1b + b1): the
    extra-part matmul collapses to a (8, Y) @ (Y, H) on-device precompute
    plus a rank-8 one-hot matmul; with K=8 the four hc-chunk matmuls run
    CONCURRENTLY in four 32-row groups of the PE array (row tiling).
  * pooling scales the 8-col one-hot by e and runs the four subtile
    matmuls concurrently in four 32-partition output groups (col tiling);
    a final select-matrix matmul folds the four groups.
  * per-segment max subtraction in the softmax is dropped (shift
    invariant, logits are O(0.1)); b3 dropped for the same reason.
  * the host ships ps twice: token-major bf16 (pool path) and
    feature-major fp8 (MLP path), pre-arranged to the exact SBUF layout:
    no PE transposes, no dtype-converting DMAs.
  * h1/h2/logits matmuls run fp8 DoubleRow (measured ~216ns per
    FD=512 matmul with LDWEIGHTS fully hidden), fp32 PSUM accumulation.
"""

import numpy as np
import ml_dtypes

import concourse.bass as bass
import concourse.mybir as mybir
import concourse.tile as tile
from concourse import bacc
from concourse.bass_utils import run_bass_kernel_spmd
from concourse.masks import make_identity

B = 64
T = 65536
X = 512
KIO = 5
Y = X * KIO          # 2560
H = 512
NCORES = 8
P = 128
FP32 = mybir.dt.float32
BF16 = mybir.dt.bfloat16
FP8 = mybir.dt.float8e4
AF = mybir.ActivationFunctionType
ALU = mybir.AluOpType

KC = X // P          # 4 contraction chunks for 512-dims
HC = H // P          # 4 output chunks for 512-dims
NKB = Y // P         # 20 contraction chunks of W1b
MT = 512             # tokens per MLP tile
NSUB = MT // P       # 128-token subtiles per MLP tile
BL = B // NCORES     # segments owned per core (local)
TLOC = 8704          # per-core padded token count (8192 + 512 slack)


def build(tloc=TLOC):
    """Build + compile the SPMD kernel for per-core token count `tloc`."""
    nt = tloc // MT
    nc = bacc.Bacc(
        "TRN2", target_bir_lowering=False, debug=False, num_devices=NCORES
    )

    psT = nc.dram_tensor("psT", [nt, P, KC, MT], FP8, kind="ExternalInput").ap()
    # ps carries the 8-col local one-hot in columns X:X+BL (one DMA/tile)
    ps = nc.dram_tensor("ps", [nt, P, NSUB, X + BL], BF16, kind="ExternalInput").ap()
    st4 = nc.dram_tensor("st4", [P, tloc], BF16, kind="ExternalInput").ap()
    ioT = nc.dram_tensor("ioT", [P, NKB, BL], BF16, kind="ExternalInput").ap()
    w1b = nc.dram_tensor("w1b", [P, NKB, H], BF16, kind="ExternalInput").ap()
    b1 = nc.dram_tensor("b1", [1, H], BF16, kind="ExternalInput").ap()
    w1a = nc.dram_tensor("w1a", [P, KC, H], FP8, kind="ExternalInput").ap()
    w2 = nc.dram_tensor("w2", [P, KC, H], FP8, kind="ExternalInput").ap()
    b2 = nc.dram_tensor("b2", [P, HC], FP32, kind="ExternalInput").ap()
    w3 = nc.dram_tensor("w3", [P, KC, 1], FP8, kind="ExternalInput").ap()
    sel = nc.dram_tensor("sel", [P, BL], BF16, kind="ExternalInput").ap()
    rep4 = nc.dram_tensor("rep4", [BL, P], BF16, kind="ExternalInput").ap()
    wf1 = nc.dram_tensor("wf1", [P, KC, H], BF16, kind="ExternalInput").ap()
    bf1_t = nc.dram_tensor("bf1", [P, HC], FP32, kind="ExternalInput").ap()
    wf2 = nc.dram_tensor("wf2", [P, KC, 2], BF16, kind="ExternalInput").ap()
    bf2_t = nc.dram_tensor("bf2", [2, 1], FP32, kind="ExternalInput").ap()
    outT = nc.dram_tensor("outT", [2, BL], FP32, kind="ExternalOutput").ap()

    with tile.TileContext(nc) as tc:
        with (
            tc.tile_pool(name="const", bufs=1) as cpool,
            tc.tile_pool(name="work", bufs=2) as wpool,
            tc.tile_pool(name="psum", bufs=1, space="PSUM") as ppool,
        ):
            # ---------------- constants ----------------
            identf = cpool.tile([1, 1], FP32)
            nc.gpsimd.memset(identf, 1.0)
            ones_b = cpool.tile([1, BL], BF16)
            nc.gpsimd.memset(ones_b, 1.0)
            ones_col = cpool.tile([P, 1], BF16)
            nc.gpsimd.memset(ones_col, 1.0)
            identbr = cpool.tile([BL, BL], FP32)
            make_identity(nc, identbr)

            # DMA issue order is tuned so the first ~10us keeps every queue
            # streaming what the PE consumes first: ps0 (pool of tile 0) on
            # sync ahead of w1b's sync share; w1b split scalar/sync feeding
            # the seg-block matmuls; st4's first chunk on gpsimd.
            def _tile_dma(j, ps_eng=None):
                psT_sb = wpool.tile(
                    [P, KC, MT], FP8, tag="psT", bufs=6, name=f"psT_{j}"
                )
                (nc.scalar if j % 2 == 0 else nc.sync).dma_start(psT_sb, psT[j])
                ps_sb = wpool.tile(
                    [P, NSUB, X + BL], BF16, tag="ps", bufs=7, name=f"ps_{j}"
                )
                (ps_eng or nc.gpsimd).dma_start(ps_sb, ps[j])
                return psT_sb, ps_sb

            ps0_sb = wpool.tile(
                [P, NSUB, X + BL], BF16, tag="ps", bufs=7, name="ps_0"
            )
            nc.gpsimd.dma_start(ps0_sb, ps[0])

            ioT_sb = cpool.tile([P, NKB, BL], BF16)
            nc.scalar.dma_start(ioT_sb, ioT)
            rep4_sb = cpool.tile([BL, P], BF16)
            nc.scalar.dma_start(rep4_sb, rep4)
            w1b_sb = wpool.tile([P, NKB, H], BF16, tag="w1b", bufs=1)
            Q = NKB // 4
            for q in range(4):
                eng = nc.scalar if q < 2 else nc.sync
                eng.dma_start(
                    w1b_sb[:, q * Q : (q + 1) * Q, :], w1b[:, q * Q : (q + 1) * Q, :]
                )
            b1_sb = cpool.tile([1, H], BF16)
            nc.scalar.dma_start(b1_sb, b1)
            w1a_sb = cpool.tile([P, KC, H], FP8)
            nc.scalar.dma_start(w1a_sb, w1a)

            psT0_sb = wpool.tile([P, KC, MT], FP8, tag="psT", bufs=6, name="psT_0")
            nc.scalar.dma_start(psT0_sb, psT[0])
            pre = [(psT0_sb, ps0_sb)]

            # PE warm-up: dependency-light matmuls on w1a (arrives ~4us)
            # fill the w1b-gated idle window 7-12us and hold HAM at 8/8;
            # result is folded into the output with weight 0 (no DCE).
            warm_psum = ppool.tile([P, H], FP32, tag="lp", bufs=1)
            NWARM = 16
            for i in range(NWARM):
                nc.tensor.matmul(
                    warm_psum,
                    w1a_sb[:, 0, 0:P],
                    w1a_sb[:, i % KC, :],
                    start=(i == 0),
                    stop=(i == NWARM - 1),
                )
            warmz_sb = wpool.tile([2, 1], FP32, tag="warmz", bufs=1)
            nc.vector.tensor_scalar_mul(warmz_sb, warm_psum[0:2, 0:1], 0.0)

            st4_sb = cpool.tile([P, tloc], BF16)
            CH = tloc // 8
            nc.sync.dma_start(st4_sb[:, 0:CH], st4[:, 0:CH])
            NPRE = min(3, nt)
            for j in range(1, NPRE):
                pre.append(_tile_dma(j))
            nc.sync.dma_start(st4_sb[:, CH : 2 * CH], st4[:, CH : 2 * CH])

            w2_sb = cpool.tile([P, KC, H], FP8)
            nc.scalar.dma_start(w2_sb, w2)
            # 16-wide w3 tile keeps the DoubleRow pair step a multiple of 16B
            w3_sb = cpool.tile([P, KC, 16], FP8)
            nc.scalar.dma_start(w3_sb[:, :, 0:1], w3)
            b2_sb = cpool.tile([P, HC], FP32)
            nc.scalar.dma_start(b2_sb, b2)
            sel_sb = cpool.tile([P, BL], BF16)
            nc.scalar.dma_start(sel_sb, sel)
            # late st4 chunks ride the gpsimd queue behind the early ps
            # tiles so they never delay psT on the HWDGE queues
            for q in range(2, 4):
                nc.gpsimd.dma_start(
                    st4_sb[:, q * CH : (q + 1) * CH], st4[:, q * CH : (q + 1) * CH]
                )
            # final-fc weights are only needed at the tail; allocate now,
            # DMA later (emitted just before the finalize section)
            wf1_sb = cpool.tile([P, KC, H], BF16)
            bf1_sb = cpool.tile([P, HC], FP32)
            wf2_sb = cpool.tile([P, KC, 2], BF16)
            bf2_sb = cpool.tile([2, 1], FP32)

            # ---------------- seg8 = io8 @ W1b + b1  (BL, H) ----------------
            seg_psum = ppool.tile([P, H], FP32, tag="poolacc", bufs=1)
            for kb in range(NKB):
                nc.tensor.matmul(
                    seg_psum[0:BL, :],
                    ioT_sb[:, kb, :],
                    w1b_sb[:, kb, :],
                    start=(kb == 0),
                    stop=False,
                )
            nc.tensor.matmul(
                seg_psum[0:BL, :], ones_b, b1_sb, start=False, stop=True
            )
            seg_sb = cpool.tile([BL, H], BF16)
            nc.vector.tensor_copy(seg_sb, seg_psum[0:BL, :])
            # replicate seg8 into the four 32-row groups via one select
            # matmul (avoids queue-blocking SBUF->SBUF DMAs)
            NR = 3 * 32 + BL  # 104 rows cover all four groups
            seg_rep = ppool.tile([P, H], FP32, tag="lp", bufs=1)
            nc.tensor.matmul(
                seg_rep[0:NR, :], rep4_sb[:, 0:NR], seg_sb, start=True, stop=True
            )
            seg_dup = cpool.tile([P, H], BF16)
            nc.vector.tensor_copy(seg_dup[0:NR, :], seg_rep[0:NR, :])

            # ---------------- main loop over MLP tiles ----------------
            # pool regions: subtile s accumulates into partitions
            # [32s, 32s+BL); a final sel-matmul folds the four regions.
            pool_psum = ppool.tile([P, H], FP32, tag="poolacc", bufs=1)
            den_psum = ppool.tile([1, NSUB * BL], FP32, tag="den", bufs=1)
            prev = None  # (j, ps_sb, e_row) of previous tile
            pending = []  # [(j, ps_sb, stmw)] awaiting pool matmuls

            def emit_e(pj, p_ps, p_erow):
                eTp = ppool.tile([P, NSUB], FP32, tag="eT", bufs=1)
                for s in range(NSUB):
                    nc.tensor.transpose(
                        eTp[:, s : s + 1],
                        p_erow[0:1, s * P : (s + 1) * P],
                        identf[0:1, 0:1],
                    )
                e_col = wpool.tile([P, NSUB], FP32, tag="ecol", bufs=2)
                nc.vector.tensor_copy(e_col, eTp)
                stmw = wpool.tile([P, NSUB, BL], BF16, tag="stmw", bufs=5)
                for s in range(NSUB):
                    nc.vector.tensor_scalar_mul(
                        stmw[:, s, :],
                        p_ps[:, s, X : X + BL],
                        e_col[:, s : s + 1],
                    )
                return stmw

            def emit_pool(pj, p_ps, stmw):
                first = pj == 0
                last = pj == nt - 1
                for s in range(NSUB):
                    nc.tensor.matmul(
                        pool_psum[32 * s : 32 * s + BL, :],
                        stmw[:, s, :],
                        p_ps[:, s, 0:X],
                        start=first,
                        stop=last,
                        tile_position=(0, 32 * s),
                        skip_group_check=True,
                    )
                nc.tensor.matmul(
                    den_psum[0:1, :],
                    ones_col,
                    stmw[:, :, :],
                    start=first,
                    stop=last,
                )

            for j in range(nt):
                psT_sb, ps_sb = pre[j] if j < NPRE else _tile_dma(j)
                if j == 4:
                    for q in range(4, 6):
                        nc.gpsimd.dma_start(
                            st4_sb[:, q * CH : (q + 1) * CH],
                            st4[:, q * CH : (q + 1) * CH],
                        )
                if j == 8:
                    for q in range(6, 8):
                        nc.gpsimd.dma_start(
                            st4_sb[:, q * CH : (q + 1) * CH],
                            st4[:, q * CH : (q + 1) * CH],
                        )
                if j == 10:
                    # final-fc weights: issued mid-loop on gpsimd so they
                    # land well before the tail without delaying any psT
                    nc.gpsimd.dma_start(wf1_sb, wf1)
                    nc.gpsimd.dma_start(bf1_sb, bf1_t)
                    nc.gpsimd.dma_start(wf2_sb, wf2)
                    nc.gpsimd.dma_start(bf2_sb, bf2_t)

                # h1 = relu(psT-major matmuls + seg8 one-hot broadcast);
                # the four K=8 seg matmuls run concurrently in four 32-row
                # groups of the PE array.
                h1_sb = wpool.tile([P, KC, MT], FP8, tag="h1", bufs=3)
                h1ps = []
                for hc in range(HC):
                    h1p = ppool.tile([P, MT], FP32, tag="hp", bufs=4)
                    for kc in range(0, KC, 2):
                        nc.tensor.matmul(
                            h1p,
                            w1a_sb[:, kc : kc + 2, hc * P : (hc + 1) * P],
                            psT_sb[:, kc : kc + 2, :],
                            start=(kc == 0),
                            stop=False,
                            perf_mode=mybir.MatmulPerfMode.DoubleRow,
                        )
                    h1ps.append(h1p)
                for hc in range(HC):
                    nc.tensor.matmul(
                        h1ps[hc],
                        seg_dup[32 * hc : 32 * hc + BL, hc * P : (hc + 1) * P],
                        st4_sb[32 * hc : 32 * hc + BL, j * MT : (j + 1) * MT],
                        start=False,
                        stop=True,
                        tile_position=(32 * hc, 0),
                    )
                for hc in range(HC):
                    if hc % 2 == 0:
                        nc.scalar.activation(h1_sb[:, hc, :], h1ps[hc], AF.Relu)
                    else:
                        nc.vector.tensor_scalar_max(h1_sb[:, hc, :], h1ps[hc], 0.0)

                # previous tile's e transpose + one-hot scaling (PE+DVE,
                # overlaps this tile's h2)
                if prev is not None:
                    pending.append((prev[0], prev[1], emit_e(prev[0], prev[1], prev[2])))
                    prev = None

                # h2
                h2_sb = wpool.tile([P, KC, MT], FP8, tag="h2", bufs=3)
                for hc in range(HC):
                    h2p = ppool.tile([P, MT], FP32, tag="hp", bufs=4)
                    for kc in range(0, KC, 2):
                        nc.tensor.matmul(
                            h2p,
                            w2_sb[:, kc : kc + 2, hc * P : (hc + 1) * P],
                            h1_sb[:, kc : kc + 2, :],
                            start=(kc == 0),
                            stop=(kc == KC - 2),
                            perf_mode=mybir.MatmulPerfMode.DoubleRow,
                        )
                    if hc % 2 == 0:
                        nc.scalar.activation(
                            h2_sb[:, hc, :], h2p, AF.Relu, bias=b2_sb[:, hc : hc + 1]
                        )
                    else:
                        nc.vector.tensor_scalar(
                            h2_sb[:, hc, :],
                            h2p,
                            b2_sb[:, hc : hc + 1],
                            0.0,
                            op0=ALU.add,
                            op1=ALU.max,
                        )

                # pooling matmuls run two tiles behind (4 col-tiled,
                # concurrent): their ps tile is guaranteed resident
                if len(pending) >= 4:
                    pj, p_ps, p_stmw = pending.pop(0)
                    emit_pool(pj, p_ps, p_stmw)

                # logits -> e = exp(logits)   (b3 dropped: cancels in softmax)
                e_row = wpool.tile([1, MT], FP32, tag="erow", bufs=2)
                lp = ppool.tile([1, MT], FP32, tag="lp", bufs=1)
                for kc in range(0, KC, 2):
                    nc.tensor.matmul(
                        lp,
                        w3_sb[:, kc : kc + 2, 0:1],
                        h2_sb[:, kc : kc + 2, :],
                        start=(kc == 0),
                        stop=(kc == KC - 2),
                        perf_mode=mybir.MatmulPerfMode.DoubleRow,
                    )
                nc.scalar.activation(e_row, lp, AF.Exp)

                prev = (j, ps_sb, e_row)

            # drain: last tile's e + remaining pools
            pending.append((prev[0], prev[1], emit_e(prev[0], prev[1], prev[2])))
            for pj, p_ps, p_stmw in pending:
                emit_pool(pj, p_ps, p_stmw)

            # ---------------- local finalize (no collectives) ----------------
            # fold the four pool regions with a select matmul
            poolc_sb = wpool.tile([P, H], BF16, tag="fin_poolc", bufs=1)
            nc.vector.tensor_copy(poolc_sb, pool_psum)
            pool8 = ppool.tile([P, H], FP32, tag="hp", bufs=4)
            nc.tensor.matmul(
                pool8[0:BL, :], sel_sb, poolc_sb, start=True, stop=True
            )
            # den: [1, NSUB*BL] -> [1, BL] (sum subtiles) -> [BL, 1]
            denr_sb = wpool.tile([1, NSUB * BL], FP32, tag="fin_denr", bufs=1)
            nc.vector.tensor_copy(denr_sb, den_psum)
            den1_sb = wpool.tile([1, BL], FP32, tag="fin_den1", bufs=1)
            nc.vector.tensor_add(
                den1_sb, denr_sb[0:1, 0:BL], denr_sb[0:1, BL : 2 * BL]
            )
            nc.vector.tensor_add(
                den1_sb, den1_sb, denr_sb[0:1, 2 * BL : 3 * BL]
            )
            nc.vector.tensor_add(
                den1_sb, den1_sb, denr_sb[0:1, 3 * BL : 4 * BL]
            )
            denTp = ppool.tile([BL, 1], FP32, tag="eT", bufs=1)
            nc.tensor.transpose(denTp, den1_sb, identf[0:1, 0:1])
            rec = wpool.tile([BL, 1], FP32, tag="fin_rec", bufs=1)
            nc.vector.reciprocal(rec, denTp)
            pooled = wpool.tile([BL, H], FP32, tag="fin_pool", bufs=1)
            nc.vector.tensor_scalar_mul(pooled, pool8[0:BL, :], rec[:, 0:1])

            # final_fc on this core's BL segment rows
            ptp = ppool.tile([P, KC * BL], FP32, tag="eT", bufs=1)
            for kc in range(KC):
                nc.tensor.transpose(
                    ptp[:, kc * BL : (kc + 1) * BL],
                    pooled[:, kc * P : (kc + 1) * P],
                    identbr,
                )
            pooledT = wpool.tile([P, KC * BL], BF16, tag="fin_poolT", bufs=1)
            nc.vector.tensor_copy(pooledT, ptp)

            hf_sb = wpool.tile([P, HC * BL], BF16, tag="fin_hf", bufs=1)
            for hc in range(HC):
                hfp = ppool.tile([P, BL], FP32, tag="hp", bufs=4)
                for kc in range(KC):
                    nc.tensor.matmul(
                        hfp,
                        wf1_sb[:, kc, hc * P : (hc + 1) * P],
                        pooledT[:, kc * BL : (kc + 1) * BL],
                        start=(kc == 0),
                        stop=(kc == KC - 1),
                    )
                nc.scalar.activation(
                    hf_sb[:, hc * BL : (hc + 1) * BL],
                    hfp,
                    AF.Relu,
                    bias=bf1_sb[:, hc : hc + 1],
                )
            op = ppool.tile([2, BL], FP32, tag="lp", bufs=1)
            for hc in range(HC):
                nc.tensor.matmul(
                    op,
                    wf2_sb[:, hc, :],
                    hf_sb[:, hc * BL : (hc + 1) * BL],
                    start=(hc == 0),
                    stop=(hc == HC - 1),
                )
            o_sb = wpool.tile([2, BL], FP32, tag="fin_o", bufs=1)
            nc.vector.tensor_scalar_add(o_sb, op, bf2_sb[:, 0:1])
            o2_sb = wpool.tile([2, BL], FP32, tag="fin_o2", bufs=1)
            nc.vector.tensor_scalar_add(o2_sb, o_sb, warmz_sb)
            nc.sync.dma_start(outT, o2_sb)

    nc.compile()
    return nc


def prep_in_maps(inputs, tloc=TLOC, ncores=NCORES):
    """Shard the full inputs into per-core input maps (host-side prep only:
    segment-aligned slicing, layout transposes, dtype casts, one-hot
    materialization, zero padding)."""
    bf = ml_dtypes.bfloat16
    f8 = ml_dtypes.float8_e4m3
    nt = tloc // MT
    ps = np.ascontiguousarray(np.asarray(inputs["ps_data"], np.float32))
    sid = np.asarray(inputs["segment_ids"], np.int64)
    io_flat = np.asarray(inputs["io_embed"], np.float32).reshape(B, -1)
    ttot = ps.shape[0]
    assert sid.shape[0] == ttot

    # segment-aligned split: core c owns all tokens of segments [8c, 8c+8)
    counts = np.bincount(sid, minlength=B)
    starts = np.zeros(B + 1, np.int64)
    np.cumsum(counts, out=starts[1:])

    W1 = np.asarray(inputs["W1"], np.float32)
    sel_host = np.zeros((P, BL), bf)
    rep4_host = np.zeros((BL, P), bf)
    for s in range(NSUB):
        for i in range(BL):
            sel_host[32 * s + i, i] = 1
            rep4_host[i, 32 * s + i] = 1

    shared = {
        "w1b": W1[X:].reshape(P, NKB, H).astype(bf),
        "b1": np.asarray(inputs["b1"], np.float32).reshape(1, H).astype(bf),
        "w1a": np.ascontiguousarray(
            W1[:X].reshape(KC, P, H).transpose(1, 0, 2)
        ).astype(f8),
        "w2": np.ascontiguousarray(
            np.asarray(inputs["W2"], np.float32).reshape(KC, P, H).transpose(1, 0, 2)
        ).astype(f8),
        "b2": np.ascontiguousarray(
            np.asarray(inputs["b2"], np.float32).reshape(HC, P).T
        ),
        "w3": np.ascontiguousarray(
            np.asarray(inputs["W3"], np.float32).reshape(KC, P, 1).transpose(1, 0, 2)
        ).astype(f8),
        "sel": sel_host,
        "rep4": rep4_host,
        "wf1": np.ascontiguousarray(
            np.asarray(inputs["Wf1"], np.float32).reshape(KC, P, H).transpose(1, 0, 2)
        ).astype(bf),
        "bf1": np.ascontiguousarray(
            np.asarray(inputs["bf1"], np.float32).reshape(HC, P).T
        ),
        "wf2": np.ascontiguousarray(
            np.asarray(inputs["Wf2"], np.float32).reshape(KC, P, 2).transpose(1, 0, 2)
        ).astype(bf),
        "bf2": np.asarray(inputs["bf2"], np.float32).reshape(2, 1).copy(),
    }
    in_maps = []
    for c in range(ncores):
        lo, hi = starts[c * BL], starts[(c + 1) * BL]
        cnt = int(hi - lo)
        assert cnt <= tloc, f"core {c} owns {cnt} tokens > tloc={tloc}"
        psc = np.zeros((tloc, X), np.float32)
        psc[:cnt] = ps[lo:hi]
        sidl = sid[lo:hi] - c * BL  # local segment ids 0..BL-1
        # feature-major fp8 for the MLP path: [nt, P, KC, MT],
        # [j, p, kc, m] = psc[j*MT + m, kc*P + p]
        psT_c = np.ascontiguousarray(
            psc.reshape(nt, MT, KC, P).transpose(0, 3, 2, 1)
        ).astype(f8)
        # token-major bf16 for the pool path with the local one-hot in the
        # last BL columns: [nt, P, NSUB, X+BL],
        # [j, p, s, x] = aug[j*MT + s*P + p, x]
        oh8 = np.zeros((tloc, BL), np.float32)
        oh8[np.arange(cnt), sidl] = 1
        aug = np.concatenate([psc, oh8], axis=1)
        ps_c = np.ascontiguousarray(
            aug.reshape(nt, NSUB, P, X + BL).transpose(0, 2, 1, 3)
        ).astype(bf)
        # st4: local one-hot transposed, replicated in the 4 row groups
        st4_c = np.zeros((P, tloc), bf)
        oh8T = oh8.astype(bf).T
        for g in range(HC):
            st4_c[32 * g : 32 * g + BL, :] = oh8T
        ioT_c = np.ascontiguousarray(
            io_flat[c * BL : (c + 1) * BL].T
        ).reshape(P, NKB, BL).astype(bf)
        in_maps.append(
            {
                "psT": psT_c,
                "ps": ps_c,
                "st4": st4_c,
                "ioT": ioT_c,
                **shared,
            }
        )
    return in_maps


_NC_CACHE = {}


def _get_nc(tloc=TLOC):
    if tloc not in _NC_CACHE:
        _NC_CACHE[tloc] = build(tloc)
    return _NC_CACHE[tloc]


def run(inputs, trace=False):
    sid = np.asarray(inputs["segment_ids"], np.int64)
    counts = np.bincount(sid, minlength=B)
    mx = int(
        max(counts[c * BL : (c + 1) * BL].sum() for c in range(NCORES))
    )
    tloc = max(TLOC, ((mx + MT - 1) // MT) * MT)
    nc = _get_nc(tloc)
    in_maps = prep_in_maps(inputs, tloc=tloc)
    res = run_bass_kernel_spmd(nc, in_maps, core_ids=list(range(NCORES)), trace=trace)
    out = np.concatenate(
        [res.results[c]["outT"].T for c in range(NCORES)], axis=0
    ).astype(np.float32)
    return np.ascontiguousarray(out), res


def kernel(**inputs):
    out, _ = run(inputs)
    return out
